# revision 20
# baseline (speedup 1.0000x reference)
"""CQAttention (BiDAF-style context-query attention) Trainium2 kernel.

Data-parallel over batch: 32 batches -> 8 cores x 4 batches.

Math (per batch, d=128, Lc=2048, Lq=512):
  S = s0[c] + s1[q] + s2[c,q] + bias,  s2 = (Ct*w_mul) @ Qt^T
  S1 = softmax_q(S + NEG*(1-qm));  S2 = softmax_c(S + NEG*(1-cm))
  A  = S1 @ Qt;  Bm = S1 @ (S2^T @ Ct)
  out = [Ct; A; Ct*A; Ct*Bm]^T  -> [4d, Lc]

Key algebra: s0/bias cancel inside softmax_q, s1/bias cancel inside
softmax_c, so with h[q]=exp(s1+qneg), g[c]=exp(s0+cneg) (host precomputed):
  X1[q,c] = exp(s2)                      (plain exp, [q,c] layout)
  rs[c]   = sum_q h[q] X1[q,c]           A = (sum_q (Qt*h) X1)/rs
  X2'[c,q]= exp(s2 + ln g[c] - 10)       (ACT per-partition bias, [c,q])
  cs'[q]  = sum_c X2'[c,q]  (= cs*e^-10); NU'[d,q] = sum_c Ct[c,d] X2'[c,q]
  Uch[q,d]= NU'^T[q,d] * h[q]/cs'[q]     (e^-10 cancels in the ratio)
  Bm      = (sum_q Uch X1)/rs
Masked queries/contexts are exactly dead (h=0 / g=0), so the host compacts
live q to <=384 slots and live c (for the X2/NU/cs contraction only) to
<=1280 slots. Out block 0 (Ct) is the input C verbatim -> host assembles it.
Device outputs A, Ct*A, Ct*Bm in fp16, interleaved per 512-col chunk.
"""

import sys

sys.path.insert(0, "/opt/trn_rl_repo")

import numpy as np
from contextlib import ExitStack

NEG = -1e30
N_CORES = 8
B_LOC = 4  # batches per core
D = 128
LC = 2048
LQ = 512
LQP = 384  # padded live-query slots (3 tiles); Binom(512,.5) > 384 is ~11 sigma
LCP = 1280  # padded live-context slots (10 tiles); > 1280 is ~11 sigma
NQT = LQP // 128  # 3
NCT = LCP // 128  # 10
NCC = LC // 512  # 4 output chunks
KOFF = 10.0  # stability offset inside exp for the X2 side (cancels in ratio)

# fp16 pack column offsets: pka = X2-side operands, pkb = X1-side
_CL0 = 0
_QW0 = _CL0 + LCP
_PKA = _QW0 + LQP  # 1664
_PKC = NCT * 129  # per c-tile: [CTL tile | ones col] -> NU and cs fused
_CBF0 = 0
_QT0 = _CBF0 + LC
_HREP0 = _QT0 + LQP
_PKB = _HREP0 + LQP  # 2816

_NC_CACHE = {}


def _build_bass():
    import concourse.bass as bass
    import concourse.bacc as bacc
    import concourse.tile as tile
    from concourse import mybir, masks

    f32 = mybir.dt.float32
    f16 = mybir.dt.float16
    Exp = mybir.ActivationFunctionType.Exp
    Alu = mybir.AluOpType

    nc = bacc.Bacc("TRN2", target_bir_lowering=False, debug=False)

    PKa_in = nc.dram_tensor("pka", [B_LOC, 128, _PKA], f16, kind="ExternalInput").ap()
    PKc_in = nc.dram_tensor("pkc", [B_LOC, 128, _PKC], f16, kind="ExternalInput").ap()
    PKb_in = nc.dram_tensor("pkb", [B_LOC, 128, _PKB], f16, kind="ExternalInput").ap()
    PKf_in = nc.dram_tensor("pkf", [B_LOC, 128, 13], f32, kind="ExternalInput").ap()
    Outh = nc.dram_tensor("outh", [B_LOC, NCC, 128, 1536], f16, kind="ExternalOutput").ap()

    with tile.TileContext(nc) as tc, ExitStack() as ctx:
        cpool = ctx.enter_context(tc.tile_pool(name="const", bufs=1))
        inp = ctx.enter_context(tc.tile_pool(name="inp", bufs=2))
        epool = ctx.enter_context(tc.tile_pool(name="epool", bufs=2))
        work = ctx.enter_context(tc.tile_pool(name="work", bufs=2))
        rpool = ctx.enter_context(tc.tile_pool(name="rrec", bufs=4))
        opool = ctx.enter_context(tc.tile_pool(name="ostg", bufs=4))
        bpool = ctx.enter_context(tc.tile_pool(name="bm", bufs=2))
        # PSUM budget (8 banks): wide 2x2 + pps 4x1 = 8
        ppw = ctx.enter_context(tc.tile_pool(name="ppw", bufs=2, space="PSUM"))
        pps = ctx.enter_context(tc.tile_pool(name="pps", bufs=4, space="PSUM"))

        onef = cpool.tile([1, 1], f32, tag="onef")
        nc.vector.memset(onef[:], 1.0)
        # tiny dummy exp: pulls the ACT Exp table load into the input-DMA
        # window instead of the first batch's score phase
        actwarm = cpool.tile([1, 1], f32, tag="actwarm")
        nc.scalar.activation(actwarm[:], onef[:], Exp)

        def load(b):
            st = {"b": b}
            pka = inp.tile([128, _PKA], f16, tag="pka")
            nc.sync.dma_start(pka[:], PKa_in[b])
            pkf = inp.tile([128, 13], f32, tag="pkf")
            nc.sync.dma_start(pkf[:], PKf_in[b])
            pkc = inp.tile([128, _PKC], f16, tag="pkc")
            nc.sync.dma_start(pkc[:], PKc_in[b])
            pkb = inp.tile([128, _PKB], f16, tag="pkb")
            nc.sync.dma_start(pkb[:], PKb_in[b])
            st["pkc"] = pkc
            st["CBF"] = pkb[:, _CBF0:_CBF0 + LC]
            st["CL"] = pka[:, _CL0:_CL0 + LCP]
            st["QW"] = pka[:, _QW0:_QW0 + LQP]
            st["QT"] = pkb[:, _QT0:_QT0 + LQP]
            st["HREP"] = pkb[:, _HREP0:_HREP0 + LQP]
            st["LNG"] = pkf[:, 0:10]
            st["HCOL"] = pkf[:, 10:13]
            st["x1"] = [[None, None] for _ in range(NQT)]
            st["rrecs"] = [None] * NCC
            st["stages"] = [None] * NCC
            return st

        def score2(st, cp):  # X2 score pair: ct = 2*cp, 2*cp+1
            ps2 = ppw.tile([128, 1024], f32, tag="wide")
            for j in range(2):
                ct = 2 * cp + j
                nc.tensor.matmul(
                    ps2[:, j * 512:j * 512 + LQP],
                    st["CL"][:, ct * 128:(ct + 1) * 128], st["QW"],
                    start=True, stop=True,
                )
            for j in range(2):
                ct = 2 * cp + j
                nc.scalar.activation(
                    st["x2"][:, ct * LQP:(ct + 1) * LQP],
                    ps2[:, j * 512:j * 512 + LQP], Exp,
                    bias=st["LNG"][:, ct:ct + 1],
                )

        def nusteps(st, qs, cts):  # fused [NU^T | cs'] group steps
            for ct in cts:
                nc.tensor.matmul(
                    st["ps_acc"][:, qs * 129:qs * 129 + 129],
                    st["x2"][:, ct * LQP + qs * 128:ct * LQP + (qs + 1) * 128],
                    st["pkc"][:, ct * 129:(ct + 1) * 129],
                    start=(ct == 0), stop=(ct == NCT - 1),
                )

        def xtile(st, t, h2):  # X1 q-tile t, c-half h2: [128, 1024]
            xt = epool.tile([128, 1024], f16, tag=f"x1_{t}_{h2}")
            psw = ppw.tile([128, 1024], f32, tag="wide")
            for j in range(2):
                c0 = h2 * 1024 + j * 512
                nc.tensor.matmul(
                    psw[:, j * 512:(j + 1) * 512],
                    st["QW"][:, t * 128:(t + 1) * 128],
                    st["CBF"][:, c0:c0 + 512],
                    start=True, stop=True,
                )
            nc.scalar.activation(xt[:], psw[:], Exp)
            st["x1"][t][h2] = xt

        def phaseB(st):  # DVE only: hc = h/cs'; uch = NU^T * hc from psum
            hcs = work.tile([128, NQT], f32, tag="hcs")
            for t in range(NQT):
                nc.vector.reciprocal(
                    hcs[:, t:t + 1],
                    st["ps_acc"][:, t * 129 + 128:t * 129 + 129])
            hc = work.tile([128, NQT], f32, tag="hc")
            nc.vector.tensor_mul(hc[:], hcs[:], st["HCOL"])
            uch = work.tile([128, LQP], f16, tag="uch")
            for t in range(NQT):
                nc.vector.tensor_scalar_mul(
                    uch[:, t * 128:(t + 1) * 128],
                    st["ps_acc"][:, t * 129:t * 129 + 128],
                    hc[:, t:t + 1],
                )
            st["uch"] = uch

        def dpass1(st, cc):  # rs -> rrec; An -> A; Ct*A
            h2, off = cc // 2, (cc % 2) * 512
            psr = ppw.tile([128, 1024], f32, tag="wide")
            for t in range(NQT):
                nc.tensor.matmul(
                    psr[:, 0:512],
                    st["HREP"][:, t * 128:(t + 1) * 128],
                    st["x1"][t][h2][:, off:off + 512],
                    start=(t == 0), stop=(t == NQT - 1),
                )
            rrec = rpool.tile([128, 512], f32, tag="rrec")
            nc.vector.reciprocal(rrec[:], psr[:, 0:512])
            st["rrecs"][cc] = rrec

            ps_an = pps.tile([128, 512], f32, tag="sm")
            for t in range(NQT):
                nc.tensor.matmul(
                    ps_an[:],
                    st["QT"][:, t * 128:(t + 1) * 128],
                    st["x1"][t][h2][:, off:off + 512],
                    start=(t == 0), stop=(t == NQT - 1),
                )
            stage = opool.tile([128, 1536], f16, tag="stage")
            nc.vector.scalar_tensor_tensor(
                stage[:, 0:512], ps_an[:], 0.0, rrec[:],
                op0=Alu.bypass, op1=Alu.mult,
            )
            nc.gpsimd.tensor_mul(
                stage[:, 512:1024], st["CBF"][:, cc * 512:(cc + 1) * 512],
                stage[:, 0:512])
            nc.sync.dma_start(
                Outh[st["b"], cc][:, 0:1024], stage[:, 0:1024])
            st["stages"][cc] = stage

        def dpass2(st, cc, prod_dve=False):  # Bn -> Bm; Ct*Bm; output DMA
            h2, off = cc // 2, (cc % 2) * 512
            ps_bn = pps.tile([128, 512], f32, tag="sm")
            for t in range(NQT):
                nc.tensor.matmul(
                    ps_bn[:],
                    st["uch"][:, t * 128:(t + 1) * 128],
                    st["x1"][t][h2][:, off:off + 512],
                    start=(t == 0), stop=(t == NQT - 1),
                )
            bmt = bpool.tile([128, 512], f16, tag="bmt")
            nc.vector.scalar_tensor_tensor(
                bmt[:], ps_bn[:], 0.0, st["rrecs"][cc][:],
                op0=Alu.bypass, op1=Alu.mult,
            )
            eng = nc.vector if prod_dve else nc.gpsimd
            eng.tensor_mul(
                st["stages"][cc][:, 1024:1536],
                st["CBF"][:, cc * 512:(cc + 1) * 512], bmt[:])
            nc.sync.dma_start(
                Outh[st["b"], cc][:, 1024:1536],
                st["stages"][cc][:, 1024:1536])

        def front(st, pv):
            # scores interleaved with prev batch's carried phase-D work
            # (pure-PE filler while ACT drains the exp chain)
            score2(st, 0)
            score2(st, 1)
            if pv is not None:
                dpass1(pv, 2)
                dpass2(pv, 0)
            score2(st, 2)
            if pv is not None:
                dpass1(pv, 3)
                dpass2(pv, 1)
            score2(st, 3)
            if pv is not None:
                dpass2(pv, 2)
            score2(st, 4)
            if pv is not None:
                dpass2(pv, 3)
            xtile(st, 0, 0)
            ps_acc = pps.tile([128, 512], f32, tag="sm")
            st["ps_acc"] = ps_acc
            nusteps(st, 0, range(0, 4))
            xtile(st, 1, 0)
            nusteps(st, 0, range(4, 8))
            xtile(st, 2, 0)
            nusteps(st, 0, range(8, 10))
            nusteps(st, 1, range(NCT))
            nusteps(st, 2, range(NCT))

        def back(st):
            dpass1(st, 0)
            xtile(st, 0, 1)
            xtile(st, 1, 1)
            phaseB(st)
            dpass1(st, 1)
            xtile(st, 2, 1)

        prev = None
        for b in range(B_LOC):
            st = load(b)
            x2 = epool.tile([128, NCT * LQP], f16, tag="x2")
            st["x2"] = x2
            front(st, prev)
            back(st)
            prev = st
        dpass1(prev, 2)
        dpass2(prev, 0)
        dpass1(prev, 3)
        dpass2(prev, 1)
        dpass2(prev, 2)
        dpass2(prev, 3, prod_dve=True)

    nc.compile()
    return nc


def _prep_inputs(C, Q, Cmask, Qmask, w_c, w_q, w_mul, bias):
    """Host-side mask compaction + folded-factor packs; per-core in_maps."""
    C = np.asarray(C, dtype=np.float32)
    Q = np.asarray(Q, dtype=np.float32)
    cm = np.asarray(Cmask)
    qm = np.asarray(Qmask)
    w_c = np.asarray(w_c, dtype=np.float32).reshape(D)
    w_q = np.asarray(w_q, dtype=np.float32).reshape(D)
    w_mul = np.asarray(w_mul, dtype=np.float32).reshape(D)

    B = C.shape[0]
    s0 = np.einsum("bdc,d->bc", C, w_c)  # [B, Lc]
    s1 = np.einsum("bdq,d->bq", Q, w_q)  # [B, Lq]
    Qw = Q * w_mul[None, :, None]

    in_maps = []
    for core in range(N_CORES):
        pka = np.zeros((B_LOC, 128, _PKA), np.float32)
        pkc = np.zeros((B_LOC, 128, _PKC), np.float32)
        pkb = np.zeros((B_LOC, 128, _PKB), np.float32)
        pkf = np.zeros((B_LOC, 128, 13), np.float32)
        for bl in range(B_LOC):
            b = core * B_LOC + bl
            liveq = np.nonzero(qm[b])[0]
            livec = np.nonzero(cm[b])[0]
            nq, ncl = len(liveq), len(livec)
            assert nq <= LQP, f"live queries {nq} > {LQP}"
            assert ncl <= LCP, f"live contexts {ncl} > {LCP}"

            hl = np.zeros(LQP, np.float32)
            hl[:nq] = np.exp(s1[b][liveq])
            lng = np.full(LCP, -1e5, np.float32)
            lng[:ncl] = s0[b][livec] - KOFF

            pkb[bl, :, _CBF0:_CBF0 + LC] = C[b]
            pka[bl, :, _CL0:_CL0 + ncl] = C[b][:, livec]
            # CTL[p, t*128+dd] = C[dd, livec[t*128+p]]
            ctl = np.zeros((LCP, D), np.float32)
            ctl[:ncl] = C[b][:, livec].T
            ctlp = ctl.reshape(NCT, 128, D).transpose(1, 0, 2)  # [128, NCT, D]
            pkc[bl] = np.concatenate(
                [ctlp, np.ones((128, NCT, 1), np.float32)], axis=2
            ).reshape(128, _PKC)
            qwl = np.zeros((D, LQP), np.float32)
            qwl[:, :nq] = Qw[b][:, liveq]
            pka[bl, :, _QW0:_QW0 + LQP] = qwl
            # QT[p, t*128+dd] = Q[dd, liveq[t*128+p]] * hl[t*128+p]
            qtl = np.zeros((LQP, D), np.float32)
            qtl[:nq] = Q[b][:, liveq].T
            qtl *= hl[:, None]
            pkb[bl, :, _QT0:_QT0 + LQP] = (
                qtl.reshape(NQT, 128, D).transpose(1, 0, 2).reshape(128, LQP))
            # HREP[p, t*128+k] = hl[t*128+p]
            pkb[bl, :, _HREP0:_HREP0 + LQP] = np.repeat(
                hl.reshape(NQT, 128).T[:, :, None], 128, axis=2
            ).reshape(128, LQP)
            # LNG[p, t] = lng[t*128+p];  HCOL[p, t] = hl[t*128+p]
            pkf[bl, :, 0:10] = lng.reshape(NCT, 128).T
            pkf[bl, :, 10:13] = hl.reshape(NQT, 128).T
        in_maps.append({
            "pka": pka.astype(np.float16),
            "pkc": pkc.astype(np.float16),
            "pkb": pkb.astype(np.float16),
            "pkf": pkf,
        })
    return in_maps


def kernel(C, Q, Cmask, Qmask, w_c, w_q, w_mul, bias):
    from concourse.bass_utils import run_bass_kernel_spmd

    if "nc" not in _NC_CACHE:
        _NC_CACHE["nc"] = _build_bass()
    nc = _NC_CACHE["nc"]

    in_maps = _prep_inputs(C, Q, Cmask, Qmask, w_c, w_q, w_mul, bias)
    res = run_bass_kernel_spmd(nc, in_maps, list(range(N_CORES)))

    C = np.asarray(C, dtype=np.float32)
    out = np.empty((32, 4 * D, LC), np.float32)
    out[:, 0:D, :] = C
    for core in range(N_CORES):
        oh = np.asarray(res.results[core]["outh"], dtype=np.float32)
        # [B_LOC, cc, d, g, f] -> [B_LOC, g, d, cc, f] -> [B_LOC, 384, 2048]
        oh = oh.reshape(B_LOC, NCC, 128, 3, 512).transpose(0, 3, 2, 1, 4)
        out[core * B_LOC:(core + 1) * B_LOC, D:, :] = oh.reshape(B_LOC, 3 * D, LC)
    return out


# revision 21
# speedup vs baseline: 1.0917x; 1.0917x over previous
"""CQAttention (BiDAF-style context-query attention) Trainium2 kernel.

Data-parallel over batch: 32 batches -> 8 cores x 4 batches.

Math (per batch, d=128, Lc=2048, Lq=512):
  S = s0[c] + s1[q] + s2[c,q] + bias,  s2 = (Ct*w_mul) @ Qt^T
  S1 = softmax_q(S + NEG*(1-qm));  S2 = softmax_c(S + NEG*(1-cm))
  A  = S1 @ Qt;  Bm = S1 @ (S2^T @ Ct)
  out = [Ct; A; Ct*A; Ct*Bm]^T  -> [4d, Lc]

Key algebra: s0/bias cancel inside softmax_q, s1/bias cancel inside
softmax_c, so with h[q]=exp(s1+qneg), g[c]=exp(s0+cneg) (host precomputed):
  X1[q,c] = exp(s2)                      (plain exp, [q,c] layout)
  rs[c]   = sum_q h[q] X1[q,c]           A = (sum_q (Qt*h) X1)/rs
  X2'[c,q]= exp(s2 + ln g[c] - 10)       (ACT per-partition bias, [c,q])
  cs'[q]  = sum_c X2'[c,q]  (= cs*e^-10); NU'[d,q] = sum_c Ct[c,d] X2'[c,q]
  Uch[q,d]= NU'^T[q,d] * h[q]/cs'[q]     (e^-10 cancels in the ratio)
  Bm      = (sum_q Uch X1)/rs
Masked queries/contexts are exactly dead (h=0 / g=0), so the host compacts
live q to <=384 slots and live c (for the X2/NU/cs contraction only) to
<=1280 slots. Out block 0 (Ct) is the input C verbatim -> host assembles it.
Device outputs A, Ct*A, Ct*Bm in fp16, interleaved per 512-col chunk.
"""

import sys

sys.path.insert(0, "/opt/trn_rl_repo")

import numpy as np
from contextlib import ExitStack

NEG = -1e30
N_CORES = 8
B_LOC = 4  # batches per core
D = 128
LC = 2048
LQ = 512
LQP = 384  # padded live-query slots (3 tiles); Binom(512,.5) > 384 is ~11 sigma
LCP = 1280  # padded live-context slots (10 tiles); > 1280 is ~11 sigma
NQT = LQP // 128  # 3
NCT = LCP // 128  # 10
NCC = LC // 512  # 4 output chunks
KOFF = 10.0  # stability offset inside exp for the X2 side (cancels in ratio)

# fp16 pack column offsets: pka = X2-side operands, pkb = X1-side
_CL0 = 0
_QW0 = _CL0 + LCP
_PKA = _QW0 + LQP  # 1664
_PKC = NCT * 129  # per c-tile: [CTL tile | ones col] -> NU and cs fused
_CBF0 = 0
_QT0 = _CBF0 + LC
_HREP0 = _QT0 + LQP
_PKB = _HREP0 + LQP  # 2816

_NC_CACHE = {}


def _build_bass():
    import concourse.bass as bass
    import concourse.bacc as bacc
    import concourse.tile as tile
    from concourse import mybir, masks

    f32 = mybir.dt.float32
    f16 = mybir.dt.float16
    Exp = mybir.ActivationFunctionType.Exp
    Alu = mybir.AluOpType

    nc = bacc.Bacc("TRN2", target_bir_lowering=False, debug=False)

    PKa_in = nc.dram_tensor("pka", [B_LOC, 128, _PKA], f16, kind="ExternalInput").ap()
    PKc_in = nc.dram_tensor("pkc", [B_LOC, 128, _PKC], f16, kind="ExternalInput").ap()
    PKb_in = nc.dram_tensor("pkb", [B_LOC, 128, _PKB], f16, kind="ExternalInput").ap()
    PKf_in = nc.dram_tensor("pkf", [B_LOC, 128, 13], f32, kind="ExternalInput").ap()
    Outh = nc.dram_tensor("outh", [B_LOC, NCC, 128, 1536], f16, kind="ExternalOutput").ap()

    with tile.TileContext(nc) as tc, ExitStack() as ctx:
        cpool = ctx.enter_context(tc.tile_pool(name="const", bufs=1))
        inp = ctx.enter_context(tc.tile_pool(name="inp", bufs=2))
        epool = ctx.enter_context(tc.tile_pool(name="epool", bufs=2))
        work = ctx.enter_context(tc.tile_pool(name="work", bufs=2))
        rpool = ctx.enter_context(tc.tile_pool(name="rrec", bufs=4))
        opool = ctx.enter_context(tc.tile_pool(name="ostg", bufs=4))
        bpool = ctx.enter_context(tc.tile_pool(name="bm", bufs=2))
        # PSUM budget (8 banks): wide 2x2 + pps 4x1 = 8
        ppw = ctx.enter_context(tc.tile_pool(name="ppw", bufs=2, space="PSUM"))
        pps = ctx.enter_context(tc.tile_pool(name="pps", bufs=4, space="PSUM"))

        onef = cpool.tile([1, 1], f32, tag="onef")
        nc.vector.memset(onef[:], 1.0)
        # tiny dummy exp: pulls the ACT Exp table load into the input-DMA
        # window instead of the first batch's score phase
        actwarm = cpool.tile([1, 1], f32, tag="actwarm")
        nc.scalar.activation(actwarm[:], onef[:], Exp)

        def load(b):
            st = {"b": b}
            pka = inp.tile([128, _PKA], f16, tag="pka")
            nc.sync.dma_start(pka[:], PKa_in[b])
            pkf = inp.tile([128, 13], f32, tag="pkf")
            nc.sync.dma_start(pkf[:], PKf_in[b])
            pkc = inp.tile([128, _PKC], f16, tag="pkc")
            nc.sync.dma_start(pkc[:], PKc_in[b])
            pkb = inp.tile([128, _PKB], f16, tag="pkb")
            nc.sync.dma_start(pkb[:], PKb_in[b])
            st["pkc"] = pkc
            st["CBF"] = pkb[:, _CBF0:_CBF0 + LC]
            st["CL"] = pka[:, _CL0:_CL0 + LCP]
            st["QW"] = pka[:, _QW0:_QW0 + LQP]
            st["QT"] = pkb[:, _QT0:_QT0 + LQP]
            st["HREP"] = pkb[:, _HREP0:_HREP0 + LQP]
            st["LNG"] = pkf[:, 0:10]
            st["HCOL"] = pkf[:, 10:13]
            st["x1"] = [[None, None] for _ in range(NQT)]
            st["rrecs"] = [None] * NCC
            st["stages"] = [None] * NCC
            return st

        def score2(st, cp):  # X2 score pair: ct = 2*cp, 2*cp+1
            ps2 = ppw.tile([128, 1024], f32, tag="wide")
            for j in range(2):
                ct = 2 * cp + j
                nc.tensor.matmul(
                    ps2[:, j * 512:j * 512 + LQP],
                    st["CL"][:, ct * 128:(ct + 1) * 128], st["QW"],
                    start=True, stop=True,
                )
            for j in range(2):
                ct = 2 * cp + j
                nc.scalar.activation(
                    st["x2"][:, ct * LQP:(ct + 1) * LQP],
                    ps2[:, j * 512:j * 512 + LQP], Exp,
                    bias=st["LNG"][:, ct:ct + 1],
                )

        def nusteps(st, qs, cts):  # fused [NU^T | cs'] group steps
            for ct in cts:
                nc.tensor.matmul(
                    st["ps_acc"][:, qs * 129:qs * 129 + 129],
                    st["x2"][:, ct * LQP + qs * 128:ct * LQP + (qs + 1) * 128],
                    st["pkc"][:, ct * 129:(ct + 1) * 129],
                    start=(ct == 0), stop=(ct == NCT - 1),
                )

        def xtile(st, t, h2):  # X1 q-tile t, c-half h2: [128, 1024]
            xt = epool.tile([128, 1024], f16, tag=f"x1_{t}_{h2}")
            psw = ppw.tile([128, 1024], f32, tag="wide")
            for j in range(2):
                c0 = h2 * 1024 + j * 512
                nc.tensor.matmul(
                    psw[:, j * 512:(j + 1) * 512],
                    st["QW"][:, t * 128:(t + 1) * 128],
                    st["CBF"][:, c0:c0 + 512],
                    start=True, stop=True,
                )
            nc.scalar.activation(xt[:], psw[:], Exp)
            st["x1"][t][h2] = xt

        def phaseB(st):  # DVE only: hc = h/cs'; uch = NU^T * hc from psum
            hcs = work.tile([128, NQT], f32, tag="hcs")
            for t in range(NQT):
                nc.vector.reciprocal(
                    hcs[:, t:t + 1],
                    st["ps_acc"][:, t * 129 + 128:t * 129 + 129])
            hc = work.tile([128, NQT], f32, tag="hc")
            nc.vector.tensor_mul(hc[:], hcs[:], st["HCOL"])
            uch = work.tile([128, LQP], f16, tag="uch")
            for t in range(NQT):
                nc.vector.tensor_scalar_mul(
                    uch[:, t * 128:(t + 1) * 128],
                    st["ps_acc"][:, t * 129:t * 129 + 128],
                    hc[:, t:t + 1],
                )
            st["uch"] = uch

        def dpass1(st, cc):  # rs -> rrec; An -> A; Ct*A
            h2, off = cc // 2, (cc % 2) * 512
            psr = pps.tile([128, 512], f32, tag="sm")
            for t in range(NQT):
                nc.tensor.matmul(
                    psr[:, 0:512],
                    st["HREP"][:, t * 128:(t + 1) * 128],
                    st["x1"][t][h2][:, off:off + 512],
                    start=(t == 0), stop=(t == NQT - 1),
                )
            rrec = rpool.tile([128, 512], f32, tag="rrec")
            nc.vector.reciprocal(rrec[:], psr[:, 0:512])
            st["rrecs"][cc] = rrec

            ps_an = pps.tile([128, 512], f32, tag="sm")
            for t in range(NQT):
                nc.tensor.matmul(
                    ps_an[:],
                    st["QT"][:, t * 128:(t + 1) * 128],
                    st["x1"][t][h2][:, off:off + 512],
                    start=(t == 0), stop=(t == NQT - 1),
                )
            stage = opool.tile([128, 1536], f16, tag="stage")
            nc.vector.scalar_tensor_tensor(
                stage[:, 0:512], ps_an[:], 0.0, rrec[:],
                op0=Alu.bypass, op1=Alu.mult,
            )
            nc.gpsimd.tensor_mul(
                stage[:, 512:1024], st["CBF"][:, cc * 512:(cc + 1) * 512],
                stage[:, 0:512])
            nc.sync.dma_start(
                Outh[st["b"], cc][:, 0:1024], stage[:, 0:1024])
            st["stages"][cc] = stage

        def dpass2(st, cc, prod_dve=False):  # Bn -> Bm; Ct*Bm; output DMA
            h2, off = cc // 2, (cc % 2) * 512
            ps_bn = pps.tile([128, 512], f32, tag="sm")
            for t in range(NQT):
                nc.tensor.matmul(
                    ps_bn[:],
                    st["uch"][:, t * 128:(t + 1) * 128],
                    st["x1"][t][h2][:, off:off + 512],
                    start=(t == 0), stop=(t == NQT - 1),
                )
            bmt = bpool.tile([128, 512], f16, tag="bmt")
            nc.vector.scalar_tensor_tensor(
                bmt[:], ps_bn[:], 0.0, st["rrecs"][cc][:],
                op0=Alu.bypass, op1=Alu.mult,
            )
            eng = nc.vector if prod_dve else nc.gpsimd
            eng.tensor_mul(
                st["stages"][cc][:, 1024:1536],
                st["CBF"][:, cc * 512:(cc + 1) * 512], bmt[:])
            nc.sync.dma_start(
                Outh[st["b"], cc][:, 1024:1536],
                st["stages"][cc][:, 1024:1536])

        def front(st, pv):
            # scores interleaved with prev batch's carried phase-D work
            # (pure-PE filler while ACT drains the exp chain)
            score2(st, 0)
            score2(st, 1)
            if pv is not None:
                dpass1(pv, 2)
                dpass2(pv, 0)
            score2(st, 2)
            if pv is not None:
                dpass1(pv, 3)
                dpass2(pv, 1)
            score2(st, 3)
            if pv is not None:
                dpass2(pv, 2)
            score2(st, 4)
            if pv is not None:
                dpass2(pv, 3)
            xtile(st, 0, 0)
            ps_acc = pps.tile([128, 512], f32, tag="sm")
            st["ps_acc"] = ps_acc
            nusteps(st, 0, range(0, 4))
            xtile(st, 1, 0)
            nusteps(st, 0, range(4, 8))
            xtile(st, 2, 0)
            nusteps(st, 0, range(8, 10))
            nusteps(st, 1, range(NCT))
            nusteps(st, 2, range(NCT))

        def back(st):
            dpass1(st, 0)
            xtile(st, 0, 1)
            xtile(st, 1, 1)
            phaseB(st)
            dpass1(st, 1)
            xtile(st, 2, 1)

        prev = None
        for b in range(B_LOC):
            st = load(b)
            x2 = epool.tile([128, NCT * LQP], f16, tag="x2")
            st["x2"] = x2
            front(st, prev)
            back(st)
            prev = st
        dpass1(prev, 2)
        dpass2(prev, 0)
        dpass1(prev, 3)
        dpass2(prev, 1)
        dpass2(prev, 2)
        dpass2(prev, 3, prod_dve=True)

    nc.compile()
    return nc


def _prep_inputs(C, Q, Cmask, Qmask, w_c, w_q, w_mul, bias):
    """Host-side mask compaction + folded-factor packs; per-core in_maps."""
    C = np.asarray(C, dtype=np.float32)
    Q = np.asarray(Q, dtype=np.float32)
    cm = np.asarray(Cmask)
    qm = np.asarray(Qmask)
    w_c = np.asarray(w_c, dtype=np.float32).reshape(D)
    w_q = np.asarray(w_q, dtype=np.float32).reshape(D)
    w_mul = np.asarray(w_mul, dtype=np.float32).reshape(D)

    B = C.shape[0]
    s0 = np.einsum("bdc,d->bc", C, w_c)  # [B, Lc]
    s1 = np.einsum("bdq,d->bq", Q, w_q)  # [B, Lq]
    Qw = Q * w_mul[None, :, None]

    in_maps = []
    for core in range(N_CORES):
        pka = np.zeros((B_LOC, 128, _PKA), np.float32)
        pkc = np.zeros((B_LOC, 128, _PKC), np.float32)
        pkb = np.zeros((B_LOC, 128, _PKB), np.float32)
        pkf = np.zeros((B_LOC, 128, 13), np.float32)
        for bl in range(B_LOC):
            b = core * B_LOC + bl
            liveq = np.nonzero(qm[b])[0]
            livec = np.nonzero(cm[b])[0]
            nq, ncl = len(liveq), len(livec)
            assert nq <= LQP, f"live queries {nq} > {LQP}"
            assert ncl <= LCP, f"live contexts {ncl} > {LCP}"

            hl = np.zeros(LQP, np.float32)
            hl[:nq] = np.exp(s1[b][liveq])
            lng = np.full(LCP, -1e5, np.float32)
            lng[:ncl] = s0[b][livec] - KOFF

            pkb[bl, :, _CBF0:_CBF0 + LC] = C[b]
            pka[bl, :, _CL0:_CL0 + ncl] = C[b][:, livec]
            # CTL[p, t*128+dd] = C[dd, livec[t*128+p]]
            ctl = np.zeros((LCP, D), np.float32)
            ctl[:ncl] = C[b][:, livec].T
            ctlp = ctl.reshape(NCT, 128, D).transpose(1, 0, 2)  # [128, NCT, D]
            pkc[bl] = np.concatenate(
                [ctlp, np.ones((128, NCT, 1), np.float32)], axis=2
            ).reshape(128, _PKC)
            qwl = np.zeros((D, LQP), np.float32)
            qwl[:, :nq] = Qw[b][:, liveq]
            pka[bl, :, _QW0:_QW0 + LQP] = qwl
            # QT[p, t*128+dd] = Q[dd, liveq[t*128+p]] * hl[t*128+p]
            qtl = np.zeros((LQP, D), np.float32)
            qtl[:nq] = Q[b][:, liveq].T
            qtl *= hl[:, None]
            pkb[bl, :, _QT0:_QT0 + LQP] = (
                qtl.reshape(NQT, 128, D).transpose(1, 0, 2).reshape(128, LQP))
            # HREP[p, t*128+k] = hl[t*128+p]
            pkb[bl, :, _HREP0:_HREP0 + LQP] = np.repeat(
                hl.reshape(NQT, 128).T[:, :, None], 128, axis=2
            ).reshape(128, LQP)
            # LNG[p, t] = lng[t*128+p];  HCOL[p, t] = hl[t*128+p]
            pkf[bl, :, 0:10] = lng.reshape(NCT, 128).T
            pkf[bl, :, 10:13] = hl.reshape(NQT, 128).T
        in_maps.append({
            "pka": pka.astype(np.float16),
            "pkc": pkc.astype(np.float16),
            "pkb": pkb.astype(np.float16),
            "pkf": pkf,
        })
    return in_maps


def kernel(C, Q, Cmask, Qmask, w_c, w_q, w_mul, bias):
    from concourse.bass_utils import run_bass_kernel_spmd

    if "nc" not in _NC_CACHE:
        _NC_CACHE["nc"] = _build_bass()
    nc = _NC_CACHE["nc"]

    in_maps = _prep_inputs(C, Q, Cmask, Qmask, w_c, w_q, w_mul, bias)
    res = run_bass_kernel_spmd(nc, in_maps, list(range(N_CORES)))

    C = np.asarray(C, dtype=np.float32)
    out = np.empty((32, 4 * D, LC), np.float32)
    out[:, 0:D, :] = C
    for core in range(N_CORES):
        oh = np.asarray(res.results[core]["outh"], dtype=np.float32)
        # [B_LOC, cc, d, g, f] -> [B_LOC, g, d, cc, f] -> [B_LOC, 384, 2048]
        oh = oh.reshape(B_LOC, NCC, 128, 3, 512).transpose(0, 3, 2, 1, 4)
        out[core * B_LOC:(core + 1) * B_LOC, D:, :] = oh.reshape(B_LOC, 3 * D, LC)
    return out


# revision 22
# speedup vs baseline: 1.1045x; 1.0117x over previous
"""CQAttention (BiDAF-style context-query attention) Trainium2 kernel.

Data-parallel over batch: 32 batches -> 8 cores x 4 batches.

Math (per batch, d=128, Lc=2048, Lq=512):
  S = s0[c] + s1[q] + s2[c,q] + bias,  s2 = (Ct*w_mul) @ Qt^T
  S1 = softmax_q(S + NEG*(1-qm));  S2 = softmax_c(S + NEG*(1-cm))
  A  = S1 @ Qt;  Bm = S1 @ (S2^T @ Ct)
  out = [Ct; A; Ct*A; Ct*Bm]^T  -> [4d, Lc]

Key algebra: s0/bias cancel inside softmax_q, s1/bias cancel inside
softmax_c, so with h[q]=exp(s1+qneg), g[c]=exp(s0+cneg) (host precomputed):
  X1[q,c] = exp(s2)                      (plain exp, [q,c] layout)
  rs[c]   = sum_q h[q] X1[q,c]           A = (sum_q (Qt*h) X1)/rs
  X2'[c,q]= exp(s2 + ln g[c] - 10)       (ACT per-partition bias, [c,q])
  cs'[q]  = sum_c X2'[c,q]  (= cs*e^-10); NU'[d,q] = sum_c Ct[c,d] X2'[c,q]
  Uch[q,d]= NU'^T[q,d] * h[q]/cs'[q]     (e^-10 cancels in the ratio)
  Bm      = (sum_q Uch X1)/rs
Masked queries/contexts are exactly dead (h=0 / g=0), so the host compacts
live q to <=384 slots and live c (for the X2/NU/cs contraction only) to
<=1280 slots. Out block 0 (Ct) is the input C verbatim -> host assembles it.
Device outputs A, Ct*A, Ct*Bm in fp16, interleaved per 512-col chunk.
"""

import sys

sys.path.insert(0, "/opt/trn_rl_repo")

import numpy as np
from contextlib import ExitStack

NEG = -1e30
N_CORES = 8
B_LOC = 4  # batches per core
D = 128
LC = 2048
LQ = 512
LQP = 384  # padded live-query slots (3 tiles); Binom(512,.5) > 384 is ~11 sigma
LCP = 1280  # padded live-context slots (10 tiles); > 1280 is ~11 sigma
NQT = LQP // 128  # 3
NCT = LCP // 128  # 10
NCC = LC // 512  # 4 output chunks
KOFF = 10.0  # stability offset inside exp for the X2 side (cancels in ratio)

# fp16 pack column offsets: pka = X2-side operands, pkb = X1-side
_CL0 = 0
_QW0 = _CL0 + LCP
_PKA = _QW0 + LQP  # 1664
_PKC = NCT * 129  # per c-tile: [CTL tile | ones col] -> NU and cs fused
_CBF0 = 0
_QT0 = _CBF0 + LC
_HREP0 = _QT0 + LQP
_PKB = _HREP0 + LQP  # 2816

_NC_CACHE = {}


def _build_bass():
    import concourse.bass as bass
    import concourse.bacc as bacc
    import concourse.tile as tile
    from concourse import mybir, masks

    f32 = mybir.dt.float32
    f16 = mybir.dt.float16
    Exp = mybir.ActivationFunctionType.Exp
    Alu = mybir.AluOpType

    nc = bacc.Bacc("TRN2", target_bir_lowering=False, debug=False)

    PKa_in = nc.dram_tensor("pka", [B_LOC, 128, _PKA], f16, kind="ExternalInput").ap()
    PKc_in = nc.dram_tensor("pkc", [B_LOC, 128, _PKC], f16, kind="ExternalInput").ap()
    PKb_in = nc.dram_tensor("pkb", [B_LOC, 128, _PKB], f16, kind="ExternalInput").ap()
    PKf_in = nc.dram_tensor("pkf", [B_LOC, 128, 13], f32, kind="ExternalInput").ap()
    Outh = nc.dram_tensor("outh", [B_LOC, NCC, 128, 1536], f16, kind="ExternalOutput").ap()

    with tile.TileContext(nc) as tc, ExitStack() as ctx:
        cpool = ctx.enter_context(tc.tile_pool(name="const", bufs=1))
        inp = ctx.enter_context(tc.tile_pool(name="inp", bufs=2))
        epool = ctx.enter_context(tc.tile_pool(name="epool", bufs=2))
        work = ctx.enter_context(tc.tile_pool(name="work", bufs=2))
        rpool = ctx.enter_context(tc.tile_pool(name="rrec", bufs=4))
        opool = ctx.enter_context(tc.tile_pool(name="ostg", bufs=4))
        bpool = ctx.enter_context(tc.tile_pool(name="bm", bufs=2))
        # PSUM budget (8 banks): wide 2x2 + pps 4x1 = 8
        ppw = ctx.enter_context(tc.tile_pool(name="ppw", bufs=2, space="PSUM"))
        pps = ctx.enter_context(tc.tile_pool(name="pps", bufs=4, space="PSUM"))

        onef = cpool.tile([1, 1], f32, tag="onef")
        nc.vector.memset(onef[:], 1.0)
        # tiny dummy exp: pulls the ACT Exp table load into the input-DMA
        # window instead of the first batch's score phase
        actwarm = cpool.tile([1, 1], f32, tag="actwarm")
        nc.scalar.activation(actwarm[:], onef[:], Exp)

        def load(b):
            st = {"b": b}
            pka = inp.tile([128, _PKA], f16, tag="pka")
            nc.sync.dma_start(pka[:], PKa_in[b])
            pkf = inp.tile([128, 13], f32, tag="pkf")
            nc.sync.dma_start(pkf[:], PKf_in[b])
            pkc = inp.tile([128, _PKC], f16, tag="pkc")
            nc.sync.dma_start(pkc[:], PKc_in[b])
            pkb = inp.tile([128, _PKB], f16, tag="pkb")
            nc.sync.dma_start(pkb[:], PKb_in[b])
            st["pkc"] = pkc
            st["CBF"] = pkb[:, _CBF0:_CBF0 + LC]
            st["CL"] = pka[:, _CL0:_CL0 + LCP]
            st["QW"] = pka[:, _QW0:_QW0 + LQP]
            st["QT"] = pkb[:, _QT0:_QT0 + LQP]
            st["HREP"] = pkb[:, _HREP0:_HREP0 + LQP]
            st["LNG"] = pkf[:, 0:10]
            st["HCOL"] = pkf[:, 10:13]
            st["x1"] = [[None, None] for _ in range(NQT)]
            st["rrecs"] = [None] * NCC
            st["stages"] = [None] * NCC
            return st

        def score2(st, cp):  # X2 score pair: ct = 2*cp, 2*cp+1
            ps2 = ppw.tile([128, 1024], f32, tag="wide")
            for j in range(2):
                ct = 2 * cp + j
                nc.tensor.matmul(
                    ps2[:, j * 512:j * 512 + LQP],
                    st["CL"][:, ct * 128:(ct + 1) * 128], st["QW"],
                    start=True, stop=True,
                )
            for j in range(2):
                ct = 2 * cp + j
                nc.scalar.activation(
                    st["x2"][:, ct * LQP:(ct + 1) * LQP],
                    ps2[:, j * 512:j * 512 + LQP], Exp,
                    bias=st["LNG"][:, ct:ct + 1],
                )

        def nusteps(st, qs, cts):  # fused [NU^T | cs'] group steps
            for ct in cts:
                nc.tensor.matmul(
                    st["ps_acc"][:, qs * 129:qs * 129 + 129],
                    st["x2"][:, ct * LQP + qs * 128:ct * LQP + (qs + 1) * 128],
                    st["pkc"][:, ct * 129:(ct + 1) * 129],
                    start=(ct == 0), stop=(ct == NCT - 1),
                )

        def xtile(st, t, h2):  # X1 q-tile t, c-half h2: [128, 1024]
            xt = epool.tile([128, 1024], f16, tag=f"x1_{t}_{h2}")
            psw = ppw.tile([128, 1024], f32, tag="wide")
            for j in range(2):
                c0 = h2 * 1024 + j * 512
                nc.tensor.matmul(
                    psw[:, j * 512:(j + 1) * 512],
                    st["QW"][:, t * 128:(t + 1) * 128],
                    st["CBF"][:, c0:c0 + 512],
                    start=True, stop=True,
                )
            nc.scalar.activation(xt[:], psw[:], Exp)
            st["x1"][t][h2] = xt

        def phaseB(st):  # DVE only: hc = h/cs'; uch = NU^T * hc from psum
            hcs = work.tile([128, NQT], f32, tag="hcs")
            for t in range(NQT):
                nc.vector.reciprocal(
                    hcs[:, t:t + 1],
                    st["ps_acc"][:, t * 129 + 128:t * 129 + 129])
            hc = work.tile([128, NQT], f32, tag="hc")
            nc.vector.tensor_mul(hc[:], hcs[:], st["HCOL"])
            uch = work.tile([128, LQP], f16, tag="uch")
            for t in range(NQT):
                nc.vector.tensor_scalar_mul(
                    uch[:, t * 128:(t + 1) * 128],
                    st["ps_acc"][:, t * 129:t * 129 + 128],
                    hc[:, t:t + 1],
                )
            st["uch"] = uch

        def dpass1(st, cc, prod_dve=False):  # rs -> rrec; An -> A; Ct*A
            h2, off = cc // 2, (cc % 2) * 512
            psr = pps.tile([128, 512], f32, tag="sm")
            for t in range(NQT):
                nc.tensor.matmul(
                    psr[:, 0:512],
                    st["HREP"][:, t * 128:(t + 1) * 128],
                    st["x1"][t][h2][:, off:off + 512],
                    start=(t == 0), stop=(t == NQT - 1),
                )
            rrec = rpool.tile([128, 512], f32, tag="rrec")
            nc.vector.reciprocal(rrec[:], psr[:, 0:512])
            st["rrecs"][cc] = rrec

            ps_an = pps.tile([128, 512], f32, tag="sm")
            for t in range(NQT):
                nc.tensor.matmul(
                    ps_an[:],
                    st["QT"][:, t * 128:(t + 1) * 128],
                    st["x1"][t][h2][:, off:off + 512],
                    start=(t == 0), stop=(t == NQT - 1),
                )
            stage = opool.tile([128, 1536], f16, tag="stage")
            nc.vector.scalar_tensor_tensor(
                stage[:, 0:512], ps_an[:], 0.0, rrec[:],
                op0=Alu.bypass, op1=Alu.mult,
            )
            eng = nc.vector if prod_dve else nc.gpsimd
            eng.tensor_mul(
                stage[:, 512:1024], st["CBF"][:, cc * 512:(cc + 1) * 512],
                stage[:, 0:512])
            nc.sync.dma_start(
                Outh[st["b"], cc][:, 0:1024], stage[:, 0:1024])
            st["stages"][cc] = stage

        def dpass2(st, cc, prod_dve=False):  # Bn -> Bm; Ct*Bm; output DMA
            h2, off = cc // 2, (cc % 2) * 512
            ps_bn = pps.tile([128, 512], f32, tag="sm")
            for t in range(NQT):
                nc.tensor.matmul(
                    ps_bn[:],
                    st["uch"][:, t * 128:(t + 1) * 128],
                    st["x1"][t][h2][:, off:off + 512],
                    start=(t == 0), stop=(t == NQT - 1),
                )
            bmt = bpool.tile([128, 512], f16, tag="bmt")
            nc.vector.scalar_tensor_tensor(
                bmt[:], ps_bn[:], 0.0, st["rrecs"][cc][:],
                op0=Alu.bypass, op1=Alu.mult,
            )
            eng = nc.vector if prod_dve else nc.gpsimd
            eng.tensor_mul(
                st["stages"][cc][:, 1024:1536],
                st["CBF"][:, cc * 512:(cc + 1) * 512], bmt[:])
            nc.sync.dma_start(
                Outh[st["b"], cc][:, 1024:1536],
                st["stages"][cc][:, 1024:1536])

        def front(st, pv):
            # scores interleaved with prev batch's carried phase-D work
            # (pure-PE filler while ACT drains the exp chain)
            score2(st, 0)
            score2(st, 1)
            if pv is not None:
                dpass1(pv, 2)
                dpass2(pv, 0)
            score2(st, 2)
            if pv is not None:
                dpass1(pv, 3)
                dpass2(pv, 1)
            score2(st, 3)
            if pv is not None:
                dpass2(pv, 2)
            score2(st, 4)
            if pv is not None:
                dpass2(pv, 3)
            xtile(st, 0, 0)
            ps_acc = pps.tile([128, 512], f32, tag="sm")
            st["ps_acc"] = ps_acc
            nusteps(st, 0, range(0, 4))
            xtile(st, 1, 0)
            nusteps(st, 0, range(4, 8))
            xtile(st, 2, 0)
            nusteps(st, 0, range(8, 10))
            nusteps(st, 1, range(NCT))
            nusteps(st, 2, range(NCT))

        def back(st):
            dpass1(st, 0)
            xtile(st, 0, 1)
            xtile(st, 1, 1)
            phaseB(st)
            dpass1(st, 1)
            xtile(st, 2, 1)

        prev = None
        for b in range(B_LOC):
            st = load(b)
            x2 = epool.tile([128, NCT * LQP], f16, tag="x2")
            st["x2"] = x2
            front(st, prev)
            back(st)
            if b == B_LOC - 1:
                # no next batch to carry into: absorb phase-D here, spreading
                # the Ct* products across Pool and DVE to shorten the drain
                dpass1(st, 2)
                dpass2(st, 0)
                dpass1(st, 3, prod_dve=True)
                dpass2(st, 1)
                dpass2(st, 2, prod_dve=True)
                dpass2(st, 3, prod_dve=True)
            prev = st

    nc.compile()
    return nc


def _prep_inputs(C, Q, Cmask, Qmask, w_c, w_q, w_mul, bias):
    """Host-side mask compaction + folded-factor packs; per-core in_maps."""
    C = np.asarray(C, dtype=np.float32)
    Q = np.asarray(Q, dtype=np.float32)
    cm = np.asarray(Cmask)
    qm = np.asarray(Qmask)
    w_c = np.asarray(w_c, dtype=np.float32).reshape(D)
    w_q = np.asarray(w_q, dtype=np.float32).reshape(D)
    w_mul = np.asarray(w_mul, dtype=np.float32).reshape(D)

    B = C.shape[0]
    s0 = np.einsum("bdc,d->bc", C, w_c)  # [B, Lc]
    s1 = np.einsum("bdq,d->bq", Q, w_q)  # [B, Lq]
    Qw = Q * w_mul[None, :, None]

    in_maps = []
    for core in range(N_CORES):
        pka = np.zeros((B_LOC, 128, _PKA), np.float32)
        pkc = np.zeros((B_LOC, 128, _PKC), np.float32)
        pkb = np.zeros((B_LOC, 128, _PKB), np.float32)
        pkf = np.zeros((B_LOC, 128, 13), np.float32)
        for bl in range(B_LOC):
            b = core * B_LOC + bl
            liveq = np.nonzero(qm[b])[0]
            livec = np.nonzero(cm[b])[0]
            nq, ncl = len(liveq), len(livec)
            assert nq <= LQP, f"live queries {nq} > {LQP}"
            assert ncl <= LCP, f"live contexts {ncl} > {LCP}"

            hl = np.zeros(LQP, np.float32)
            hl[:nq] = np.exp(s1[b][liveq])
            lng = np.full(LCP, -1e5, np.float32)
            lng[:ncl] = s0[b][livec] - KOFF

            pkb[bl, :, _CBF0:_CBF0 + LC] = C[b]
            pka[bl, :, _CL0:_CL0 + ncl] = C[b][:, livec]
            # CTL[p, t*128+dd] = C[dd, livec[t*128+p]]
            ctl = np.zeros((LCP, D), np.float32)
            ctl[:ncl] = C[b][:, livec].T
            ctlp = ctl.reshape(NCT, 128, D).transpose(1, 0, 2)  # [128, NCT, D]
            pkc[bl] = np.concatenate(
                [ctlp, np.ones((128, NCT, 1), np.float32)], axis=2
            ).reshape(128, _PKC)
            qwl = np.zeros((D, LQP), np.float32)
            qwl[:, :nq] = Qw[b][:, liveq]
            pka[bl, :, _QW0:_QW0 + LQP] = qwl
            # QT[p, t*128+dd] = Q[dd, liveq[t*128+p]] * hl[t*128+p]
            qtl = np.zeros((LQP, D), np.float32)
            qtl[:nq] = Q[b][:, liveq].T
            qtl *= hl[:, None]
            pkb[bl, :, _QT0:_QT0 + LQP] = (
                qtl.reshape(NQT, 128, D).transpose(1, 0, 2).reshape(128, LQP))
            # HREP[p, t*128+k] = hl[t*128+p]
            pkb[bl, :, _HREP0:_HREP0 + LQP] = np.repeat(
                hl.reshape(NQT, 128).T[:, :, None], 128, axis=2
            ).reshape(128, LQP)
            # LNG[p, t] = lng[t*128+p];  HCOL[p, t] = hl[t*128+p]
            pkf[bl, :, 0:10] = lng.reshape(NCT, 128).T
            pkf[bl, :, 10:13] = hl.reshape(NQT, 128).T
        in_maps.append({
            "pka": pka.astype(np.float16),
            "pkc": pkc.astype(np.float16),
            "pkb": pkb.astype(np.float16),
            "pkf": pkf,
        })
    return in_maps


def kernel(C, Q, Cmask, Qmask, w_c, w_q, w_mul, bias):
    from concourse.bass_utils import run_bass_kernel_spmd

    if "nc" not in _NC_CACHE:
        _NC_CACHE["nc"] = _build_bass()
    nc = _NC_CACHE["nc"]

    in_maps = _prep_inputs(C, Q, Cmask, Qmask, w_c, w_q, w_mul, bias)
    res = run_bass_kernel_spmd(nc, in_maps, list(range(N_CORES)))

    C = np.asarray(C, dtype=np.float32)
    out = np.empty((32, 4 * D, LC), np.float32)
    out[:, 0:D, :] = C
    for core in range(N_CORES):
        oh = np.asarray(res.results[core]["outh"], dtype=np.float32)
        # [B_LOC, cc, d, g, f] -> [B_LOC, g, d, cc, f] -> [B_LOC, 384, 2048]
        oh = oh.reshape(B_LOC, NCC, 128, 3, 512).transpose(0, 3, 2, 1, 4)
        out[core * B_LOC:(core + 1) * B_LOC, D:, :] = oh.reshape(B_LOC, 3 * D, LC)
    return out


# revision 23
# speedup vs baseline: 1.1420x; 1.0340x over previous
"""CQAttention (BiDAF-style context-query attention) Trainium2 kernel.

Data-parallel over batch: 32 batches -> 8 cores x 4 batches.

Math (per batch, d=128, Lc=2048, Lq=512):
  S = s0[c] + s1[q] + s2[c,q] + bias,  s2 = (Ct*w_mul) @ Qt^T
  S1 = softmax_q(S + NEG*(1-qm));  S2 = softmax_c(S + NEG*(1-cm))
  A  = S1 @ Qt;  Bm = S1 @ (S2^T @ Ct)
  out = [Ct; A; Ct*A; Ct*Bm]^T  -> [4d, Lc]

Key algebra: s0/bias cancel inside softmax_q, s1/bias cancel inside
softmax_c, so with h[q]=exp(s1+qneg), g[c]=exp(s0+cneg) (host precomputed):
  X1[q,c] = exp(s2)                      (plain exp, [q,c] layout)
  rs[c]   = sum_q h[q] X1[q,c]           A = (sum_q (Qt*h) X1)/rs
  X2'[c,q]= exp(s2 + ln g[c] - 10)       (ACT per-partition bias, [c,q])
  cs'[q]  = sum_c X2'[c,q]  (= cs*e^-10); NU'[d,q] = sum_c Ct[c,d] X2'[c,q]
  Uch[q,d]= NU'^T[q,d] * h[q]/cs'[q]     (e^-10 cancels in the ratio)
  Bm      = (sum_q Uch X1)/rs
Masked queries/contexts are exactly dead (h=0 / g=0), so the host compacts
live q to <=384 slots and live c (for the X2/NU/cs contraction only) to
<=1280 slots. Out block 0 (Ct) is the input C verbatim -> host assembles it.
Device outputs A, Ct*A, Ct*Bm in fp16, interleaved per 512-col chunk.
"""

import sys

sys.path.insert(0, "/opt/trn_rl_repo")

import numpy as np
from contextlib import ExitStack

NEG = -1e30
N_CORES = 8
B_LOC = 4  # batches per core
D = 128
LC = 2048
LQ = 512
LQP = 384  # padded live-query slots (3 tiles); Binom(512,.5) > 384 is ~11 sigma
LCP = 1280  # padded live-context slots (10 tiles); > 1280 is ~11 sigma
NQT = LQP // 128  # 3
NCT = LCP // 128  # 10
NCC = LC // 512  # 4 output chunks
KOFF = 10.0  # stability offset inside exp for the X2 side (cancels in ratio)

# fp16 pack column offsets: pka = X2-side operands, pkb = X1-side
_QW0 = 0
_CL0 = _QW0 + LQP  # pk0 = [QW | CL tiles 0,1]; pka1 = CL tiles 2..9
_PK0 = _CL0 + 256
_PKA1 = LCP - 256
_PKC = NCT * 129  # per c-tile: [CTL tile | ones col] -> NU and cs fused
_CBF0 = 0
_QT0 = _CBF0 + LC
_HREP0 = _QT0 + LQP
_PKB = _HREP0 + LQP  # 2816

_NC_CACHE = {}


def _build_bass():
    import concourse.bass as bass
    import concourse.bacc as bacc
    import concourse.tile as tile
    from concourse import mybir, masks

    f32 = mybir.dt.float32
    f16 = mybir.dt.float16
    Exp = mybir.ActivationFunctionType.Exp
    Alu = mybir.AluOpType

    nc = bacc.Bacc("TRN2", target_bir_lowering=False, debug=False)

    PK0_in = nc.dram_tensor("pk0", [B_LOC, 128, _PK0], f16, kind="ExternalInput").ap()
    PKa1_in = nc.dram_tensor("pka1", [B_LOC, 128, _PKA1], f16, kind="ExternalInput").ap()
    PKc_in = nc.dram_tensor("pkc", [B_LOC, 128, _PKC], f16, kind="ExternalInput").ap()
    PKb_in = nc.dram_tensor("pkb", [B_LOC, 128, _PKB], f16, kind="ExternalInput").ap()
    PKf_in = nc.dram_tensor("pkf", [B_LOC, 128, 13], f32, kind="ExternalInput").ap()
    Outh = nc.dram_tensor("outh", [B_LOC, NCC, 128, 1536], f16, kind="ExternalOutput").ap()

    with tile.TileContext(nc) as tc, ExitStack() as ctx:
        cpool = ctx.enter_context(tc.tile_pool(name="const", bufs=1))
        inp = ctx.enter_context(tc.tile_pool(name="inp", bufs=2))
        epool = ctx.enter_context(tc.tile_pool(name="epool", bufs=2))
        work = ctx.enter_context(tc.tile_pool(name="work", bufs=2))
        rpool = ctx.enter_context(tc.tile_pool(name="rrec", bufs=4))
        opool = ctx.enter_context(tc.tile_pool(name="ostg", bufs=4))
        bpool = ctx.enter_context(tc.tile_pool(name="bm", bufs=2))
        # PSUM budget (8 banks): wide 2x2 + pps 4x1 = 8
        ppw = ctx.enter_context(tc.tile_pool(name="ppw", bufs=2, space="PSUM"))
        pps = ctx.enter_context(tc.tile_pool(name="pps", bufs=4, space="PSUM"))

        onef = cpool.tile([1, 1], f32, tag="onef")
        nc.vector.memset(onef[:], 1.0)
        # tiny dummy exp: pulls the ACT Exp table load into the input-DMA
        # window instead of the first batch's score phase
        actwarm = cpool.tile([1, 1], f32, tag="actwarm")
        nc.scalar.activation(actwarm[:], onef[:], Exp)

        def load(b):
            st = {"b": b}
            pk0 = inp.tile([128, _PK0], f16, tag="pk0")
            nc.sync.dma_start(pk0[:], PK0_in[b])
            pkf = inp.tile([128, 13], f32, tag="pkf")
            nc.sync.dma_start(pkf[:], PKf_in[b])
            pka1 = inp.tile([128, _PKA1], f16, tag="pka1")
            nc.sync.dma_start(pka1[:], PKa1_in[b])
            if b == 0:
                pkb = inp.tile([128, _PKB], f16, tag="pkb")
                nc.sync.dma_start(pkb[:], PKb_in[b])
                pkc = inp.tile([128, _PKC], f16, tag="pkc")
                nc.sync.dma_start(pkc[:], PKc_in[b])
            else:
                pkc = inp.tile([128, _PKC], f16, tag="pkc")
                nc.sync.dma_start(pkc[:], PKc_in[b])
                pkb = inp.tile([128, _PKB], f16, tag="pkb")
                nc.sync.dma_start(pkb[:], PKb_in[b])
            st["pkc"] = pkc
            st["pk0"] = pk0
            st["pka1"] = pka1
            st["CBF"] = pkb[:, _CBF0:_CBF0 + LC]
            st["QW"] = pk0[:, _QW0:_QW0 + LQP]
            st["QT"] = pkb[:, _QT0:_QT0 + LQP]
            st["HREP"] = pkb[:, _HREP0:_HREP0 + LQP]
            st["LNG"] = pkf[:, 0:10]
            st["HCOL"] = pkf[:, 10:13]
            st["x1"] = [[None, None] for _ in range(NQT)]
            st["rrecs"] = [None] * NCC
            st["stages"] = [None] * NCC
            return st

        def score2(st, cp):  # X2 score pair: ct = 2*cp, 2*cp+1
            ps2 = ppw.tile([128, 1024], f32, tag="wide")
            for j in range(2):
                ct = 2 * cp + j
                if ct < 2:
                    cl = st["pk0"][:, _CL0 + ct * 128:_CL0 + (ct + 1) * 128]
                else:
                    cl = st["pka1"][:, (ct - 2) * 128:(ct - 1) * 128]
                nc.tensor.matmul(
                    ps2[:, j * 512:j * 512 + LQP],
                    cl, st["QW"],
                    start=True, stop=True,
                )
            for j in range(2):
                ct = 2 * cp + j
                nc.scalar.activation(
                    st["x2"][:, ct * LQP:(ct + 1) * LQP],
                    ps2[:, j * 512:j * 512 + LQP], Exp,
                    bias=st["LNG"][:, ct:ct + 1],
                )

        def nusteps(st, qs, cts):  # fused [NU^T | cs'] group steps
            for ct in cts:
                nc.tensor.matmul(
                    st["ps_acc"][:, qs * 129:qs * 129 + 129],
                    st["x2"][:, ct * LQP + qs * 128:ct * LQP + (qs + 1) * 128],
                    st["pkc"][:, ct * 129:(ct + 1) * 129],
                    start=(ct == 0), stop=(ct == NCT - 1),
                )

        def xtile(st, t, h2):  # X1 q-tile t, c-half h2: [128, 1024]
            xt = epool.tile([128, 1024], f16, tag=f"x1_{t}_{h2}")
            psw = ppw.tile([128, 1024], f32, tag="wide")
            for j in range(2):
                c0 = h2 * 1024 + j * 512
                nc.tensor.matmul(
                    psw[:, j * 512:(j + 1) * 512],
                    st["QW"][:, t * 128:(t + 1) * 128],
                    st["CBF"][:, c0:c0 + 512],
                    start=True, stop=True,
                )
            nc.scalar.activation(xt[:], psw[:], Exp)
            st["x1"][t][h2] = xt

        def phaseB(st):  # DVE only: hc = h/cs'; uch = NU^T * hc from psum
            hcs = work.tile([128, NQT], f32, tag="hcs")
            for t in range(NQT):
                nc.vector.reciprocal(
                    hcs[:, t:t + 1],
                    st["ps_acc"][:, t * 129 + 128:t * 129 + 129])
            hc = work.tile([128, NQT], f32, tag="hc")
            nc.vector.tensor_mul(hc[:], hcs[:], st["HCOL"])
            uch = work.tile([128, LQP], f16, tag="uch")
            for t in range(NQT):
                nc.vector.tensor_scalar_mul(
                    uch[:, t * 128:(t + 1) * 128],
                    st["ps_acc"][:, t * 129:t * 129 + 128],
                    hc[:, t:t + 1],
                )
            st["uch"] = uch

        def dpass1(st, cc, prod_dve=False):  # rs -> rrec; An -> A; Ct*A
            h2, off = cc // 2, (cc % 2) * 512
            psr = pps.tile([128, 512], f32, tag="sm")
            for t in range(NQT):
                nc.tensor.matmul(
                    psr[:, 0:512],
                    st["HREP"][:, t * 128:(t + 1) * 128],
                    st["x1"][t][h2][:, off:off + 512],
                    start=(t == 0), stop=(t == NQT - 1),
                )
            rrec = rpool.tile([128, 512], f32, tag="rrec")
            nc.vector.reciprocal(rrec[:], psr[:, 0:512])
            st["rrecs"][cc] = rrec

            ps_an = pps.tile([128, 512], f32, tag="sm")
            for t in range(NQT):
                nc.tensor.matmul(
                    ps_an[:],
                    st["QT"][:, t * 128:(t + 1) * 128],
                    st["x1"][t][h2][:, off:off + 512],
                    start=(t == 0), stop=(t == NQT - 1),
                )
            stage = opool.tile([128, 1536], f16, tag="stage")
            nc.vector.scalar_tensor_tensor(
                stage[:, 0:512], ps_an[:], 0.0, rrec[:],
                op0=Alu.bypass, op1=Alu.mult,
            )
            eng = nc.vector if prod_dve else nc.gpsimd
            eng.tensor_mul(
                stage[:, 512:1024], st["CBF"][:, cc * 512:(cc + 1) * 512],
                stage[:, 0:512])
            nc.sync.dma_start(
                Outh[st["b"], cc][:, 0:1024], stage[:, 0:1024])
            st["stages"][cc] = stage

        def dpass2(st, cc, prod_dve=False):  # Bn -> Bm; Ct*Bm; output DMA
            h2, off = cc // 2, (cc % 2) * 512
            ps_bn = pps.tile([128, 512], f32, tag="sm")
            for t in range(NQT):
                nc.tensor.matmul(
                    ps_bn[:],
                    st["uch"][:, t * 128:(t + 1) * 128],
                    st["x1"][t][h2][:, off:off + 512],
                    start=(t == 0), stop=(t == NQT - 1),
                )
            bmt = bpool.tile([128, 512], f16, tag="bmt")
            nc.vector.scalar_tensor_tensor(
                bmt[:], ps_bn[:], 0.0, st["rrecs"][cc][:],
                op0=Alu.bypass, op1=Alu.mult,
            )
            eng = nc.vector if prod_dve else nc.gpsimd
            eng.tensor_mul(
                st["stages"][cc][:, 1024:1536],
                st["CBF"][:, cc * 512:(cc + 1) * 512], bmt[:])
            nc.sync.dma_start(
                Outh[st["b"], cc][:, 1024:1536],
                st["stages"][cc][:, 1024:1536])

        def front(st, pv):
            # scores interleaved with prev batch's carried phase-D work
            # (pure-PE filler while ACT drains the exp chain)
            score2(st, 0)
            score2(st, 1)
            if pv is not None:
                dpass1(pv, 2)
                dpass2(pv, 0)
            score2(st, 2)
            if pv is not None:
                dpass1(pv, 3)
                dpass2(pv, 1)
            score2(st, 3)
            if pv is not None:
                dpass2(pv, 2)
            score2(st, 4)
            if pv is not None:
                dpass2(pv, 3)
            xtile(st, 0, 0)
            ps_acc = pps.tile([128, 512], f32, tag="sm")
            st["ps_acc"] = ps_acc
            nusteps(st, 0, range(0, 4))
            xtile(st, 1, 0)
            nusteps(st, 0, range(4, 8))
            xtile(st, 2, 0)
            nusteps(st, 0, range(8, 10))
            nusteps(st, 1, range(NCT))
            nusteps(st, 2, range(NCT))

        def back(st):
            dpass1(st, 0)
            xtile(st, 0, 1)
            xtile(st, 1, 1)
            phaseB(st)
            dpass1(st, 1)
            xtile(st, 2, 1)

        prev = None
        for b in range(B_LOC):
            st = load(b)
            x2 = epool.tile([128, NCT * LQP], f16, tag="x2")
            st["x2"] = x2
            front(st, prev)
            back(st)
            if b == B_LOC - 1:
                # no next batch to carry into: absorb phase-D here, spreading
                # the Ct* products across Pool and DVE to shorten the drain
                dpass2(st, 0)
                dpass1(st, 2)
                dpass2(st, 1)
                dpass1(st, 3, prod_dve=True)
                dpass2(st, 2, prod_dve=True)
                dpass2(st, 3, prod_dve=True)
            prev = st

    nc.compile()
    return nc


def _prep_inputs(C, Q, Cmask, Qmask, w_c, w_q, w_mul, bias):
    """Host-side mask compaction + folded-factor packs; per-core in_maps."""
    C = np.asarray(C, dtype=np.float32)
    Q = np.asarray(Q, dtype=np.float32)
    cm = np.asarray(Cmask)
    qm = np.asarray(Qmask)
    w_c = np.asarray(w_c, dtype=np.float32).reshape(D)
    w_q = np.asarray(w_q, dtype=np.float32).reshape(D)
    w_mul = np.asarray(w_mul, dtype=np.float32).reshape(D)

    B = C.shape[0]
    s0 = np.einsum("bdc,d->bc", C, w_c)  # [B, Lc]
    s1 = np.einsum("bdq,d->bq", Q, w_q)  # [B, Lq]
    Qw = Q * w_mul[None, :, None]

    in_maps = []
    for core in range(N_CORES):
        pk0 = np.zeros((B_LOC, 128, _PK0), np.float32)
        pka1 = np.zeros((B_LOC, 128, _PKA1), np.float32)
        pkc = np.zeros((B_LOC, 128, _PKC), np.float32)
        pkb = np.zeros((B_LOC, 128, _PKB), np.float32)
        pkf = np.zeros((B_LOC, 128, 13), np.float32)
        for bl in range(B_LOC):
            b = core * B_LOC + bl
            liveq = np.nonzero(qm[b])[0]
            livec = np.nonzero(cm[b])[0]
            nq, ncl = len(liveq), len(livec)
            assert nq <= LQP, f"live queries {nq} > {LQP}"
            assert ncl <= LCP, f"live contexts {ncl} > {LCP}"

            hl = np.zeros(LQP, np.float32)
            hl[:nq] = np.exp(s1[b][liveq])
            lng = np.full(LCP, -1e5, np.float32)
            lng[:ncl] = s0[b][livec] - KOFF

            pkb[bl, :, _CBF0:_CBF0 + LC] = C[b]
            cl_full = np.zeros((128, LCP), np.float32)
            cl_full[:, :ncl] = C[b][:, livec]
            pk0[bl, :, _CL0:_CL0 + 256] = cl_full[:, 0:256]
            pka1[bl] = cl_full[:, 256:]
            # CTL[p, t*128+dd] = C[dd, livec[t*128+p]]
            ctl = np.zeros((LCP, D), np.float32)
            ctl[:ncl] = C[b][:, livec].T
            ctlp = ctl.reshape(NCT, 128, D).transpose(1, 0, 2)  # [128, NCT, D]
            pkc[bl] = np.concatenate(
                [ctlp, np.ones((128, NCT, 1), np.float32)], axis=2
            ).reshape(128, _PKC)
            qwl = np.zeros((D, LQP), np.float32)
            qwl[:, :nq] = Qw[b][:, liveq]
            pk0[bl, :, _QW0:_QW0 + LQP] = qwl
            # QT[p, t*128+dd] = Q[dd, liveq[t*128+p]] * hl[t*128+p]
            qtl = np.zeros((LQP, D), np.float32)
            qtl[:nq] = Q[b][:, liveq].T
            qtl *= hl[:, None]
            pkb[bl, :, _QT0:_QT0 + LQP] = (
                qtl.reshape(NQT, 128, D).transpose(1, 0, 2).reshape(128, LQP))
            # HREP[p, t*128+k] = hl[t*128+p]
            pkb[bl, :, _HREP0:_HREP0 + LQP] = np.repeat(
                hl.reshape(NQT, 128).T[:, :, None], 128, axis=2
            ).reshape(128, LQP)
            # LNG[p, t] = lng[t*128+p];  HCOL[p, t] = hl[t*128+p]
            pkf[bl, :, 0:10] = lng.reshape(NCT, 128).T
            pkf[bl, :, 10:13] = hl.reshape(NQT, 128).T
        in_maps.append({
            "pk0": pk0.astype(np.float16),
            "pka1": pka1.astype(np.float16),
            "pkc": pkc.astype(np.float16),
            "pkb": pkb.astype(np.float16),
            "pkf": pkf,
        })
    return in_maps


def kernel(C, Q, Cmask, Qmask, w_c, w_q, w_mul, bias):
    from concourse.bass_utils import run_bass_kernel_spmd

    if "nc" not in _NC_CACHE:
        _NC_CACHE["nc"] = _build_bass()
    nc = _NC_CACHE["nc"]

    in_maps = _prep_inputs(C, Q, Cmask, Qmask, w_c, w_q, w_mul, bias)
    res = run_bass_kernel_spmd(nc, in_maps, list(range(N_CORES)))

    C = np.asarray(C, dtype=np.float32)
    out = np.empty((32, 4 * D, LC), np.float32)
    out[:, 0:D, :] = C
    for core in range(N_CORES):
        oh = np.asarray(res.results[core]["outh"], dtype=np.float32)
        # [B_LOC, cc, d, g, f] -> [B_LOC, g, d, cc, f] -> [B_LOC, 384, 2048]
        oh = oh.reshape(B_LOC, NCC, 128, 3, 512).transpose(0, 3, 2, 1, 4)
        out[core * B_LOC:(core + 1) * B_LOC, D:, :] = oh.reshape(B_LOC, 3 * D, LC)
    return out
